# revision 2
# baseline (speedup 1.0000x reference)
"""Binarized 2-layer conv net (BinaryConv2d -> BinaryTanh -> BinaryConv2d -> Scale)
for Trainium2, data-parallel over the batch dim across 8 NeuronCores.

Math (matching the reference):
    h   = conv2d(x, sign(w1), pad=1) + sign(b1)
    h   = sign(h)                       # sign(clip(h,-1,1)) == sign(h)
    out = (conv2d(h, sign(w2), pad=1) + sign(b2)) * scale

Device mapping (per core, 8 images):
  * x split on host into fp16 hi + fp16 lo (~24 mantissa bits together,
    effectively fp32-exact), pre-padded to 66x66.
  * x load: ONE DMA per precision per image builds the 3 dy-shifted
    overlapping slabs [96, 64*66] directly from HBM via a raw strided AP
    [[66,3],[4356,32],[1,4224]] (overlapping windows; single trigger).
  * conv1: K=96 (3 dy-slabs x 32 cin), dx taps as free-dim offsets.
    Round r computes g0 block r (PE col tile 0) and g1 block r+4 (col
    tile 1); 12 matmuls (3 dx x 2 precisions x 2 col tiles) accumulate
    into one [128,512] PSUM bank; sign(conv1+b1) evacuates in a single
    full-lane ScalarE ACT into the h slab.
  * h layout: big contiguous slab [128, 34*66] bf16. Partitions 0:64
    (g0) hold image rows -1..32 as slab rows 0..33; partitions 64:128
    (g1) hold rows 31..64. Slab row 0 of g0 / 33 of g1 are zero pad
    (memset once); only TWO halo DMAs per image: g0 row 33 <- g1 row 1
    (after round 0) and g1 row 0 <- g0 row 32 (after round 3).
  * conv2: bf16, K=64, 4 concurrent 64x64 PE tiles (2 row groups x 2
    col groups) process 4 blocks at once; 9 tap-matmuls each; dy order
    arranged so halo-dependent taps come last. PSUM evac (out = psum +
    sign(b2), exact small ints in bf16) via ScalarE ACT (pc) and DVE
    tensor_scalar (pd) into a [128, 2048] staging buffer; ONE out DMA
    per image with a 5-dim strided dst AP.
  * DMA triggers: x prefetch on the sync queue (runs ahead, nbuf=3),
    halo + out triggers on the scalar queue; ~5 triggers/image total.
"""

import numpy as np
import ml_dtypes

import bass_rust
import concourse.bass as bass
import concourse.mybir as mybir
import concourse.tile as tile
from concourse import bacc
from concourse.bass_utils import run_bass_kernel_spmd

F32 = mybir.dt.float32
F16 = mybir.dt.float16
BF16 = mybir.dt.bfloat16

N_CORES = 8
IMGS_PER_CORE = 8
CIN, COUT = 32, 64
H = W = 64
WP = 66                    # padded width
T1_FREE = H * WP           # 4224, one dy-block slab
HS_ROWS = 34               # h slab rows per partition group (img rows -1..32 / 31..64)
HS_FREE = HS_ROWS * WP     # 2244
ACT_SIGN = mybir.ActivationFunctionType.Sign
ACT_IDENT = mybir.ActivationFunctionType.Identity
ALU_ADD = mybir.AluOpType.add


def _raw_ap(tensor_handle, offset, dims):
    """Build an AP with explicit [stride, size] dims (allows overlap)."""
    return bass_rust.AP(tensor=tensor_handle, offset=offset,
                        ap=[[s, n] for s, n in dims])


def build_nc(reps: int = 1, nbuf: int = 3, pa_bufs: int = 2,
             hbufs: int = 2) -> bacc.Bacc:
    nc = bacc.Bacc("TRN2", target_bir_lowering=False)

    xhi_t = nc.dram_tensor("xhi", [IMGS_PER_CORE, CIN, WP, WP], F16, kind="ExternalInput")
    xlo_t = nc.dram_tensor("xlo", [IMGS_PER_CORE, CIN, WP, WP], F16, kind="ExternalInput")
    w1s_t = nc.dram_tensor("w1s", [128, 192], F16, kind="ExternalInput")
    w2s_t = nc.dram_tensor("w2s", [128, 576], BF16, kind="ExternalInput")
    b1s_t = nc.dram_tensor("b1s", [128, 1], F32, kind="ExternalInput")
    b2s_t = nc.dram_tensor("b2s", [128, 1], F32, kind="ExternalInput")
    out_t = nc.dram_tensor("out", [IMGS_PER_CORE, COUT, H, W], BF16, kind="ExternalOutput")

    xhi_h = xhi_t.ap().tensor
    xlo_h = xlo_t.ap().tensor
    out_h = out_t.ap().tensor
    IMG_X = CIN * WP * WP          # elements per image in xhi/xlo
    IMG_O = COUT * H * W           # elements per image in out

    with tile.TileContext(nc) as tc:
        # ---- persistent SBUF tensors ----
        w1 = nc.alloc_sbuf_tensor("w1sb", [128, 192], F16).ap()
        w2 = nc.alloc_sbuf_tensor("w2sb", [128, 576], BF16).ap()
        b1 = nc.alloc_sbuf_tensor("b1sb", [128, 1], F32).ap()
        b2 = nc.alloc_sbuf_tensor("b2sb", [128, 1], F32).ap()
        t1 = [
            [nc.alloc_sbuf_tensor(f"t1_{p}_{b}", [96, T1_FREE], F16).ap()
             for b in range(nbuf)]
            for p in range(2)
        ]
        hs = [nc.alloc_sbuf_tensor(f"hs_{b}", [128, HS_FREE], BF16).ap()
              for b in range(hbufs)]
        ob = [nc.alloc_sbuf_tensor(f"ob_{b}", [128, 2048], BF16).ap()
              for b in range(2)]

        nc.sync.dma_start(out=w1, in_=w1s_t.ap())
        nc.sync.dma_start(out=w2, in_=w2s_t.ap())
        nc.sync.dma_start(out=b1, in_=b1s_t.ap())
        nc.sync.dma_start(out=b2, in_=b2s_t.ap())
        # h slab borders (col 0/65 of every row, g0 row 0, g1 row 33) must
        # stay zero forever; per-image writes only touch interior cols 1:65
        # of rows 1..32 plus the two halo rows.
        for b in range(hbufs):
            nc.gpsimd.memset(hs[b][:, :], 0.0)

        # conv2 tap orders: halo-dependent taps last.
        # pc (g0): block 3 dy=2 needs g0 slab row 33 (halo A, after round 0).
        # pd (g1): block 4.. dy=0 needs g1 slab row 0 (halo B, after round 3).
        TAPS_PC = [(dy, dx) for dy in (0, 1, 2) for dx in range(3)]
        TAPS_PD = [(dy, dx) for dy in (1, 2, 0) for dx in range(3)]

        with tc.tile_pool(name="psA", bufs=pa_bufs, space="PSUM") as pool_a, \
             tc.tile_pool(name="psB", bufs=2, space="PSUM") as pool_cd:
            for img_v in range(IMGS_PER_CORE * reps):
                img = img_v % IMGS_PER_CORE
                tb = img_v % nbuf
                hb = img_v % hbufs
                t1h, t1l = t1[0][tb], t1[1][tb]

                # ---- x load: one strided DMA per precision (sync queue) ----
                for tt, th in ((t1h, xhi_h), (t1l, xlo_h)):
                    src = _raw_ap(th, img * IMG_X,
                                  [(WP, 3), (WP * WP, CIN), (1, T1_FREE)])
                    nc.sync.dma_start(out=tt[:, :], in_=src)

                tvh = t1h.rearrange("p (h w) -> p h w", w=WP)
                tvl = t1l.rearrange("p (h w) -> p h w", w=WP)
                hv = hs[hb].rearrange("p (r w) -> p r w", w=WP)

                # ---- conv1: 4 rounds x (2 col tiles), K=96 ----
                for r in range(4):
                    pa = pool_a.tile([128, 512], F32, tag="pa")
                    n_mm = 0
                    for dx in range(3):
                        lw = w1[0:96, dx * 64:(dx + 1) * 64]
                        for tv in (tvh, tvl):
                            st = n_mm == 0
                            sp = n_mm == 5
                            nc.tensor.matmul(
                                pa[0:64, :], lw,
                                tv[0:96, 8 * r: 8 * r + 8, dx: dx + 64],
                                start=st, stop=sp, tile_position=(0, 0))
                            nc.tensor.matmul(
                                pa[64:128, :], lw,
                                tv[0:96, 32 + 8 * r: 40 + 8 * r, dx: dx + 64],
                                start=st, stop=sp, tile_position=(0, 64))
                            n_mm += 1
                    # h = sign(conv1 + b1): g0 block r / g1 block r+4 land at
                    # the same slab-row offsets -> one full-lane ACT.
                    nc.scalar.activation(
                        out=hv[:, 1 + 8 * r: 9 + 8 * r, 1:65],
                        in_=pa[:, :].rearrange("p (a b) -> p a b", b=64),
                        func=ACT_SIGN, bias=b1[:, 0:1])
                    if r == 0:
                        # halo A: g0 slab row 33 (img row 32) <- g1 slab row 1
                        nc.scalar.dma_start(out=hv[0:64, 33:34, 1:65],
                                            in_=hv[64:128, 1:2, 1:65])
                    if r == 3:
                        # halo B: g1 slab row 0 (img row 31) <- g0 slab row 32
                        nc.scalar.dma_start(out=hv[64:128, 0:1, 1:65],
                                            in_=hv[0:64, 32:33, 1:65])

                # ---- conv2: 2 super-rounds x 4 concurrent 64x64 tiles ----
                obt = ob[img_v % 2]
                for s2 in range(2):
                    pc = pool_cd.tile([128, 512], F32, tag="pc")
                    pd = pool_cd.tile([128, 512], F32, tag="pd")
                    bA, bB = 2 * s2, 2 * s2 + 1
                    for ti in range(9):
                        st = ti == 0
                        sp = ti == 8
                        dyc, dxc = TAPS_PC[ti]
                        dyd, dxd = TAPS_PD[ti]
                        lwc = w2[0:64, (dyc * 3 + dxc) * 64:(dyc * 3 + dxc + 1) * 64]
                        lwd = w2[64:128, (dyd * 3 + dxd) * 64:(dyd * 3 + dxd + 1) * 64]
                        nc.tensor.matmul(
                            pc[0:64, :], lwc,
                            hv[0:64, 8 * bA + dyc: 8 * bA + dyc + 8, dxc: dxc + 64],
                            start=st, stop=sp, tile_position=(0, 0))
                        nc.tensor.matmul(
                            pc[64:128, :], lwc,
                            hv[0:64, 8 * bB + dyc: 8 * bB + dyc + 8, dxc: dxc + 64],
                            start=st, stop=sp, tile_position=(0, 64))
                        nc.tensor.matmul(
                            pd[0:64, :], lwd,
                            hv[64:128, 8 * bA + dyd: 8 * bA + dyd + 8, dxd: dxd + 64],
                            start=st, stop=sp, tile_position=(64, 0))
                        nc.tensor.matmul(
                            pd[64:128, :], lwd,
                            hv[64:128, 8 * bB + dyd: 8 * bB + dyd + 8, dxd: dxd + 64],
                            start=st, stop=sp, tile_position=(64, 64))
                    # out = psum + sign(b2): pc on ScalarE, pd on DVE.
                    # obt free layout: [g=2 (1024), s2=2 (512), px (1)]
                    nc.scalar.activation(
                        out=obt[:, s2 * 512:(s2 + 1) * 512], in_=pc[:, :],
                        func=ACT_IDENT, bias=b2[:, 0:1])
                    nc.vector.tensor_scalar(
                        out=obt[:, 1024 + s2 * 512: 1536 + s2 * 512],
                        in0=pd[:, :],
                        scalar1=b2[:, 0:1], scalar2=None, op0=ALU_ADD)

                # ---- one out DMA per image (scalar queue) ----
                # src: [128 partitions, (g,s,px)]; dst HBM out[img]:
                # offset(p_hi, ch, g, s, px) =
                #   p_hi*512 + ch*4096 + g*2048 + s*1024 + px
                dst = _raw_ap(out_h, img * IMG_O,
                              [(512, 2), (4096, 64), (2048, 2), (1024, 2), (1, 512)])
                nc.scalar.dma_start(out=dst, in_=obt[:, :])

    nc.compile()
    return nc


_CACHE: dict = {}


def _get_nc(reps: int = 1, **kw) -> bacc.Bacc:
    key = (reps, tuple(sorted(kw.items())))
    if key not in _CACHE:
        _CACHE[key] = build_nc(reps, **kw)
    return _CACHE[key]


def _sign(a: np.ndarray) -> np.ndarray:
    return np.where(a >= 0, np.float32(1.0), np.float32(-1.0))


def _prep_inputs(x, w1, b1, w2, b2, scale_val):
    x = np.asarray(x, np.float32)
    n = x.shape[0]
    # fp16 hi/lo split (hi+lo carries ~24 mantissa bits of x)
    xhi = x.astype(np.float16)
    xlo = (x - xhi.astype(np.float32)).astype(np.float16)
    xhi_pad = np.zeros((n, CIN, WP, WP), np.float16)
    xlo_pad = np.zeros((n, CIN, WP, WP), np.float16)
    xhi_pad[:, :, 1:65, 1:65] = xhi
    xlo_pad[:, :, 1:65, 1:65] = xlo

    w1b = _sign(np.asarray(w1, np.float32))           # [64o, 32c, 3, 3]
    w2b = _sign(np.asarray(w2, np.float32))           # [64o, 64c, 3, 3]
    w1s = np.zeros((128, 192), np.float16)
    for dx in range(3):
        for dy in range(3):
            w1s[dy * 32:dy * 32 + 32, dx * 64:(dx + 1) * 64] = w1b[:, :, dy, dx].T
    w2s = np.zeros((128, 576), ml_dtypes.bfloat16)
    for dy in range(3):
        for dx in range(3):
            tap = dy * 3 + dx
            blk = w2b[:, :, dy, dx].T.astype(ml_dtypes.bfloat16)
            w2s[0:64, tap * 64:(tap + 1) * 64] = blk
            w2s[64:128, tap * 64:(tap + 1) * 64] = blk
    b1s = np.tile(_sign(np.asarray(b1, np.float32)), 2).reshape(128, 1).astype(np.float32)
    b2s = np.tile(_sign(np.asarray(b2, np.float32)), 2).reshape(128, 1).astype(np.float32)

    per = n // N_CORES
    in_maps = []
    for i in range(N_CORES):
        sl = slice(i * per, (i + 1) * per)
        in_maps.append({
            "xhi": np.ascontiguousarray(xhi_pad[sl]),
            "xlo": np.ascontiguousarray(xlo_pad[sl]),
            "w1s": w1s, "w2s": w2s, "b1s": b1s, "b2s": b2s,
        })
    return in_maps


def kernel(x, w1, b1, w2, b2, scale) -> np.ndarray:
    scale_val = float(np.asarray(scale).reshape(-1)[0])
    nc = _get_nc(reps=1)
    in_maps = _prep_inputs(x, w1, b1, w2, b2, scale_val)
    res = run_bass_kernel_spmd(nc, in_maps, core_ids=list(range(N_CORES)))
    # out HBM layout: [img][ch][4096] where block b = pixels 512b..512b+511,
    # block = g*4 + s*2 + p_hi; pixel px of block b = image pixel b*512+px.
    out = np.concatenate([r["out"] for r in res.results], axis=0)
    return out.astype(np.float32) * np.float32(scale_val)


if __name__ == "__main__":
    rng = np.random.default_rng(0)
    ins = {
        "x": rng.standard_normal((64, 32, 64, 64), dtype=np.float32),
        "w1": (rng.standard_normal((64, 32, 3, 3)) * 0.05).astype(np.float32),
        "b1": (rng.standard_normal((64,)) * 0.05).astype(np.float32),
        "w2": (rng.standard_normal((64, 64, 3, 3)) * 0.05).astype(np.float32),
        "b2": (rng.standard_normal((64,)) * 0.05).astype(np.float32),
        "scale": np.array([0.001], np.float32),
    }
    out = kernel(**ins)
    print("out", out.shape, out.dtype, float(np.abs(out).mean()))


# revision 8
# speedup vs baseline: 1.4466x; 1.4466x over previous
"""Binarized 2-layer conv net (BinaryConv2d -> BinaryTanh -> BinaryConv2d -> Scale)
for Trainium2, data-parallel over the batch dim across 8 NeuronCores.

Math (matching the reference):
    h   = conv2d(x, sign(w1), pad=1) + sign(b1)
    h   = sign(h)                       # sign(clip(h,-1,1)) == sign(h)
    out = (conv2d(h, sign(w2), pad=1) + sign(b2)) * scale

Device mapping (per core, 8 images):
  * x split on host into fp16 hi + fp16 lo (~24 mantissa bits together,
    effectively fp32-exact), pre-padded to 66x66.
  * x load: ONE DMA per precision per image builds the 3 dy-shifted
    overlapping slabs [96, 64*66] directly from HBM via a raw strided AP
    [[66,3],[4356,32],[1,4224]] (overlapping windows; single trigger).
  * conv1: K=96 (3 dy-slabs x 32 cin), dx taps as free-dim offsets.
    Round r computes g0 block r (PE col tile 0) and g1 block r+4 (col
    tile 1); 12 matmuls (3 dx x 2 precisions x 2 col tiles) accumulate
    into one [128,512] PSUM bank; sign(conv1+b1) evacuates in a single
    full-lane ScalarE ACT into the h slab.
  * h layout: big contiguous slab [128, 34*66] bf16. Partitions 0:64
    (g0) hold image rows -1..32 as slab rows 0..33; partitions 64:128
    (g1) hold rows 31..64. Slab row 0 of g0 / 33 of g1 are zero pad
    (memset once); only TWO halo DMAs per image: g0 row 33 <- g1 row 1
    (after round 0) and g1 row 0 <- g0 row 32 (after round 3).
  * conv2: bf16, K=64, 4 concurrent 64x64 PE tiles (2 row groups x 2
    col groups) process 4 blocks at once; 9 tap-matmuls each; dy order
    arranged so halo-dependent taps come last. PSUM evac (out = psum +
    sign(b2), exact small ints in bf16) via two DVE tensor_scalar ops
    into a [128, 2048] staging buffer; ONE out DMA per image.
  * HW-DGE descriptor generation costs ~13ns/descriptor ON the
    triggering engine queue, so descriptor count (= partition count per
    DMA here) is the real currency: out HBM layout is partition-major
    [128, 8, 2048] (128 descs/img, host reorders to NCHW), x loads are
    96 descs each, halos 64 each.
  * Queues: x prefetch (issued 2 images ahead, nbuf=3) + halos on the
    sync queue; conv1 sign-ACTs + out trigger on the scalar queue.
"""

import numpy as np
import ml_dtypes

import bass_rust
import concourse.bass as bass
import concourse.mybir as mybir
import concourse.tile as tile
from concourse import bacc
from concourse.bass_utils import run_bass_kernel_spmd

F32 = mybir.dt.float32
F16 = mybir.dt.float16
BF16 = mybir.dt.bfloat16

N_CORES = 8
IMGS_PER_CORE = 8
CIN, COUT = 32, 64
H = W = 64
WP = 66                    # padded width
T1_FREE = H * WP           # 4224, one dy-block slab
HS_ROWS = 34               # h slab rows per partition group (img rows -1..32 / 31..64)
HS_FREE = HS_ROWS * WP     # 2244
ACT_SIGN = mybir.ActivationFunctionType.Sign
ACT_IDENT = mybir.ActivationFunctionType.Identity
ALU_ADD = mybir.AluOpType.add


def _raw_ap(tensor_handle, offset, dims):
    """Build an AP with explicit [stride, size] dims (allows overlap)."""
    return bass_rust.AP(tensor=tensor_handle, offset=offset,
                        ap=[[s, n] for s, n in dims])


def build_nc(reps: int = 1, nbuf: int = 3, pa_bufs: int = 2,
             hbufs: int = 2) -> bacc.Bacc:
    nc = bacc.Bacc("TRN2", target_bir_lowering=False)

    xhi_t = nc.dram_tensor("xhi", [IMGS_PER_CORE, CIN, WP, WP], F16, kind="ExternalInput")
    xlo_t = nc.dram_tensor("xlo", [IMGS_PER_CORE, CIN, WP, WP], F16, kind="ExternalInput")
    w1s_t = nc.dram_tensor("w1s", [128, 192], F16, kind="ExternalInput")
    w2s_t = nc.dram_tensor("w2s", [128, 576], BF16, kind="ExternalInput")
    b1s_t = nc.dram_tensor("b1s", [128, 1], F32, kind="ExternalInput")
    b2s_t = nc.dram_tensor("b2s", [128, 1], F32, kind="ExternalInput")
    # partition-major output: [128 sbuf partitions, img, (g, s2, px)]
    out_t = nc.dram_tensor("out", [128, IMGS_PER_CORE, 2048], BF16, kind="ExternalOutput")

    xhi_h = xhi_t.ap().tensor
    xlo_h = xlo_t.ap().tensor
    outr = out_t.ap()
    IMG_X = CIN * WP * WP          # elements per image in xhi/xlo

    with tile.TileContext(nc) as tc:
        # ---- persistent SBUF tensors ----
        w1 = nc.alloc_sbuf_tensor("w1sb", [128, 192], F16).ap()
        w2 = nc.alloc_sbuf_tensor("w2sb", [128, 576], BF16).ap()
        b1 = nc.alloc_sbuf_tensor("b1sb", [128, 1], F32).ap()
        b2 = nc.alloc_sbuf_tensor("b2sb", [128, 1], F32).ap()
        t1 = [
            [nc.alloc_sbuf_tensor(f"t1_{p}_{b}", [96, T1_FREE], F16).ap()
             for b in range(nbuf)]
            for p in range(2)
        ]
        hs = [nc.alloc_sbuf_tensor(f"hs_{b}", [128, HS_FREE], BF16).ap()
              for b in range(hbufs)]
        ob = [nc.alloc_sbuf_tensor(f"ob_{b}", [128, 2048], BF16).ap()
              for b in range(2)]

        nc.sync.dma_start(out=w1, in_=w1s_t.ap())
        nc.sync.dma_start(out=w2, in_=w2s_t.ap())
        nc.sync.dma_start(out=b1, in_=b1s_t.ap())
        nc.sync.dma_start(out=b2, in_=b2s_t.ap())
        # h slab borders (col 0/65 of every row, g0 row 0, g1 row 33) must
        # stay zero forever; per-image writes only touch interior cols 1:65
        # of rows 1..32 plus the two halo rows.
        for b in range(hbufs):
            nc.gpsimd.memset(hs[b][:, :], 0.0)

        # conv2 tap orders: halo-dependent taps last.
        # pc (g0): block 3 dy=2 needs g0 slab row 33 (halo A, after round 0).
        # pd (g1): block 4.. dy=0 needs g1 slab row 0 (halo B, after round 3).
        TAPS_PC = [(dy, dx) for dy in (0, 1, 2) for dx in range(3)]
        TAPS_PD = [(dy, dx) for dy in (1, 2, 0) for dx in range(3)]

        with tc.tile_pool(name="psA", bufs=pa_bufs, space="PSUM") as pool_a, \
             tc.tile_pool(name="psB", bufs=2, space="PSUM") as pool_cd:
            n_iters = IMGS_PER_CORE * reps

            def issue_x_load(iv):
                # one strided DMA per precision builds all 3 dy slabs
                im = iv % IMGS_PER_CORE
                for tt, th in ((t1[0][iv % nbuf], xhi_h),
                               (t1[1][iv % nbuf], xlo_h)):
                    src = _raw_ap(th, im * IMG_X,
                                  [(WP, 3), (WP * WP, CIN), (1, T1_FREE)])
                    nc.sync.dma_start(out=tt[:, :], in_=src)

            # prefetch depth 2
            issue_x_load(0)
            if n_iters > 1:
                issue_x_load(1)

            for img_v in range(n_iters):
                img = img_v % IMGS_PER_CORE
                tb = img_v % nbuf
                hb = img_v % hbufs
                t1h, t1l = t1[0][tb], t1[1][tb]
                if img_v + 2 < n_iters:
                    issue_x_load(img_v + 2)

                tvh = t1h.rearrange("p (h w) -> p h w", w=WP)
                tvl = t1l.rearrange("p (h w) -> p h w", w=WP)
                hv = hs[hb].rearrange("p (r w) -> p r w", w=WP)

                # ---- conv1: 4 rounds x (2 col tiles), K=96 ----
                for r in range(4):
                    pa = pool_a.tile([128, 512], F32, tag="pa")
                    n_mm = 0
                    for dx in range(3):
                        lw = w1[0:96, dx * 64:(dx + 1) * 64]
                        for tv in (tvh, tvl):
                            st = n_mm == 0
                            sp = n_mm == 5
                            nc.tensor.matmul(
                                pa[0:64, :], lw,
                                tv[0:96, 8 * r: 8 * r + 8, dx: dx + 64],
                                start=st, stop=sp, tile_position=(0, 0))
                            nc.tensor.matmul(
                                pa[64:128, :], lw,
                                tv[0:96, 32 + 8 * r: 40 + 8 * r, dx: dx + 64],
                                start=st, stop=sp, tile_position=(0, 64))
                            n_mm += 1
                    # h = sign(conv1 + b1): g0 block r / g1 block r+4 land at
                    # the same slab-row offsets -> one full-lane ACT.
                    nc.scalar.activation(
                        out=hv[:, 1 + 8 * r: 9 + 8 * r, 1:65],
                        in_=pa[:, :].rearrange("p (a b) -> p a b", b=64),
                        func=ACT_SIGN, bias=b1[:, 0:1])
                    if r == 0:
                        # halo A: g0 slab row 33 (img row 32) <- g1 slab row 1
                        nc.sync.dma_start(out=hv[0:64, 33:34, 1:65],
                                          in_=hv[64:128, 1:2, 1:65])
                    if r == 3:
                        # halo B: g1 slab row 0 (img row 31) <- g0 slab row 32
                        nc.sync.dma_start(out=hv[64:128, 0:1, 1:65],
                                          in_=hv[0:64, 32:33, 1:65])

                # ---- conv2: 2 super-rounds x 4 concurrent 64x64 tiles ----
                obt = ob[img_v % 2]
                for s2 in range(2):
                    pc = pool_cd.tile([128, 512], F32, tag="pc")
                    pd = pool_cd.tile([128, 512], F32, tag="pd")
                    bA, bB = 2 * s2, 2 * s2 + 1
                    for ti in range(9):
                        st = ti == 0
                        sp = ti == 8
                        dyc, dxc = TAPS_PC[ti]
                        dyd, dxd = TAPS_PD[ti]
                        lwc = w2[0:64, (dyc * 3 + dxc) * 64:(dyc * 3 + dxc + 1) * 64]
                        lwd = w2[64:128, (dyd * 3 + dxd) * 64:(dyd * 3 + dxd + 1) * 64]
                        nc.tensor.matmul(
                            pc[0:64, :], lwc,
                            hv[0:64, 8 * bA + dyc: 8 * bA + dyc + 8, dxc: dxc + 64],
                            start=st, stop=sp, tile_position=(0, 0))
                        nc.tensor.matmul(
                            pc[64:128, :], lwc,
                            hv[0:64, 8 * bB + dyc: 8 * bB + dyc + 8, dxc: dxc + 64],
                            start=st, stop=sp, tile_position=(0, 64))
                        nc.tensor.matmul(
                            pd[0:64, :], lwd,
                            hv[64:128, 8 * bA + dyd: 8 * bA + dyd + 8, dxd: dxd + 64],
                            start=st, stop=sp, tile_position=(64, 0))
                        nc.tensor.matmul(
                            pd[64:128, :], lwd,
                            hv[64:128, 8 * bB + dyd: 8 * bB + dyd + 8, dxd: dxd + 64],
                            start=st, stop=sp, tile_position=(64, 64))
                    # out = psum + sign(b2), both halves on DVE.
                    # obt free layout: [g=2 (1024), s2=2 (512), px (1)]
                    nc.vector.tensor_scalar(
                        out=obt[:, s2 * 512:(s2 + 1) * 512], in0=pc[:, :],
                        scalar1=b2[:, 0:1], scalar2=None, op0=ALU_ADD)
                    nc.vector.tensor_scalar(
                        out=obt[:, 1024 + s2 * 512: 1536 + s2 * 512],
                        in0=pd[:, :],
                        scalar1=b2[:, 0:1], scalar2=None, op0=ALU_ADD)

                # ---- one out DMA per image (scalar queue, 128 descs) ----
                nc.scalar.dma_start(out=outr[:, img, :], in_=obt[:, :])

    nc.compile()
    return nc


_CACHE: dict = {}


def _get_nc(reps: int = 1, **kw) -> bacc.Bacc:
    key = (reps, tuple(sorted(kw.items())))
    if key not in _CACHE:
        _CACHE[key] = build_nc(reps, **kw)
    return _CACHE[key]


def _sign(a: np.ndarray) -> np.ndarray:
    return np.where(a >= 0, np.float32(1.0), np.float32(-1.0))


def _prep_inputs(x, w1, b1, w2, b2, scale_val):
    x = np.asarray(x, np.float32)
    n = x.shape[0]
    # fp16 hi/lo split (hi+lo carries ~24 mantissa bits of x)
    xhi = x.astype(np.float16)
    xlo = (x - xhi.astype(np.float32)).astype(np.float16)
    xhi_pad = np.zeros((n, CIN, WP, WP), np.float16)
    xlo_pad = np.zeros((n, CIN, WP, WP), np.float16)
    xhi_pad[:, :, 1:65, 1:65] = xhi
    xlo_pad[:, :, 1:65, 1:65] = xlo

    w1b = _sign(np.asarray(w1, np.float32))           # [64o, 32c, 3, 3]
    w2b = _sign(np.asarray(w2, np.float32))           # [64o, 64c, 3, 3]
    w1s = np.zeros((128, 192), np.float16)
    for dx in range(3):
        for dy in range(3):
            w1s[dy * 32:dy * 32 + 32, dx * 64:(dx + 1) * 64] = w1b[:, :, dy, dx].T
    w2s = np.zeros((128, 576), ml_dtypes.bfloat16)
    for dy in range(3):
        for dx in range(3):
            tap = dy * 3 + dx
            blk = w2b[:, :, dy, dx].T.astype(ml_dtypes.bfloat16)
            w2s[0:64, tap * 64:(tap + 1) * 64] = blk
            w2s[64:128, tap * 64:(tap + 1) * 64] = blk
    b1s = np.tile(_sign(np.asarray(b1, np.float32)), 2).reshape(128, 1).astype(np.float32)
    b2s = np.tile(_sign(np.asarray(b2, np.float32)), 2).reshape(128, 1).astype(np.float32)

    per = n // N_CORES
    in_maps = []
    for i in range(N_CORES):
        sl = slice(i * per, (i + 1) * per)
        in_maps.append({
            "xhi": np.ascontiguousarray(xhi_pad[sl]),
            "xlo": np.ascontiguousarray(xlo_pad[sl]),
            "w1s": w1s, "w2s": w2s, "b1s": b1s, "b2s": b2s,
        })
    return in_maps


def kernel(x, w1, b1, w2, b2, scale) -> np.ndarray:
    scale_val = float(np.asarray(scale).reshape(-1)[0])
    nc = _get_nc(reps=1)
    in_maps = _prep_inputs(x, w1, b1, w2, b2, scale_val)
    res = run_bass_kernel_spmd(nc, in_maps, core_ids=list(range(N_CORES)))
    # out HBM layout: [128, img, 2048] where partition p = (p_hi, ch),
    # free = (g, s, px); image block index = g*4 + s*2 + p_hi, image
    # pixel = block*512 + px (row-major 64x64).
    parts = []
    for r in res.results:
        o = np.asarray(r["out"]).reshape(2, 64, IMGS_PER_CORE, 2, 2, 512)
        o = o.transpose(2, 1, 3, 4, 0, 5).reshape(IMGS_PER_CORE, COUT, H, W)
        parts.append(o)
    out = np.concatenate(parts, axis=0)
    return out.astype(np.float32) * np.float32(scale_val)


if __name__ == "__main__":
    rng = np.random.default_rng(0)
    ins = {
        "x": rng.standard_normal((64, 32, 64, 64), dtype=np.float32),
        "w1": (rng.standard_normal((64, 32, 3, 3)) * 0.05).astype(np.float32),
        "b1": (rng.standard_normal((64,)) * 0.05).astype(np.float32),
        "w2": (rng.standard_normal((64, 64, 3, 3)) * 0.05).astype(np.float32),
        "b2": (rng.standard_normal((64,)) * 0.05).astype(np.float32),
        "scale": np.array([0.001], np.float32),
    }
    out = kernel(**ins)
    print("out", out.shape, out.dtype, float(np.abs(out).mean()))


# revision 17
# speedup vs baseline: 1.4726x; 1.0180x over previous
"""Binarized 2-layer conv net (BinaryConv2d -> BinaryTanh -> BinaryConv2d -> Scale)
for Trainium2, data-parallel over the batch dim across 8 NeuronCores.

Math (matching the reference):
    h   = conv2d(x, sign(w1), pad=1) + sign(b1)
    h   = sign(h)                       # sign(clip(h,-1,1)) == sign(h)
    out = (conv2d(h, sign(w2), pad=1) + sign(b2)) * scale

Device mapping (per core, 8 images):
  * x split on host into fp16 hi + fp16 lo (~24 mantissa bits together,
    effectively fp32-exact), pre-padded to 66x66.
  * x load: ONE DMA per precision per image builds the 3 dy-shifted
    overlapping slabs [96, 64*66] directly from HBM via a raw strided AP
    [[66,3],[4356,32],[1,4224]] (overlapping windows; single trigger).
  * conv1: K=96 (3 dy-slabs x 32 cin), dx taps as free-dim offsets.
    Round r computes g0 block r (PE col tile 0) and g1 block r+4 (col
    tile 1); 12 matmuls (3 dx x 2 precisions x 2 col tiles) accumulate
    into one [128,512] PSUM bank; sign(conv1+b1) evacuates in a single
    full-lane ScalarE ACT into the h slab.
  * h layout: big contiguous slab [128, 34*66] bf16. Partitions 0:64
    (g0) hold image rows -1..32 as slab rows 0..33; partitions 64:128
    (g1) hold rows 31..64. Slab rows 0 (g0/g1) and 33 are zero pad or
    unwritten; NO halo DMAs: the two cross-group boundary rows are
    consumed by conv2 directly from the other partition group via
    split tap matmuls (main 7-row N=448 + cross-group 1-row N=64 with
    the transposed tile_position). This keeps DMA entirely off the
    PE-critical path.
  * conv2: bf16, K=64, 4 concurrent 64x64 PE tiles (2 row groups x 2
    col groups) process 4 blocks at once; 9 tap-matmuls each; dy order
    arranged so boundary taps come last. PSUM evac (out = psum +
    sign(b2), exact small ints in bf16) via two DVE tensor_scalar ops
    into a [128, 2048] staging buffer; ONE out DMA per image.
  * HW-DGE descriptor generation costs ~13ns/descriptor ON the
    triggering engine queue and stalls on ring backpressure behind
    earlier bursts, so: descriptor count is the currency (out HBM
    layout is partition-major [128, 8, 2048] = 128 descs/img, host
    reorders to NCHW; x loads are 96 descs each), and ALL DMA triggers
    live on the sync queue (x prefetch issued 2 images ahead, nbuf=3,
    then out drain) where a stall only delays prefetched/buffered
    work. The scalar queue runs only the critical conv1 sign-ACTs.
"""

import numpy as np
import ml_dtypes

import bass_rust
import concourse.bass as bass
import concourse.mybir as mybir
import concourse.tile as tile
from concourse import bacc
from concourse.bass_utils import run_bass_kernel_spmd

F32 = mybir.dt.float32
F16 = mybir.dt.float16
BF16 = mybir.dt.bfloat16

N_CORES = 8
IMGS_PER_CORE = 8
CIN, COUT = 32, 64
H = W = 64
WP = 66                    # padded width
T1_FREE = H * WP           # 4224, one dy-block slab
HS_ROWS = 34               # h slab rows per partition group (img rows -1..32 / 31..64)
HS_FREE = HS_ROWS * WP     # 2244
ACT_SIGN = mybir.ActivationFunctionType.Sign
ACT_IDENT = mybir.ActivationFunctionType.Identity
ALU_ADD = mybir.AluOpType.add


def _raw_ap(tensor_handle, offset, dims):
    """Build an AP with explicit [stride, size] dims (allows overlap)."""
    return bass_rust.AP(tensor=tensor_handle, offset=offset,
                        ap=[[s, n] for s, n in dims])


def build_nc(reps: int = 1, nbuf: int = 3, pa_bufs: int = 2,
             hbufs: int = 2, bmm: bool = True, outq: str = "sync") -> bacc.Bacc:
    nc = bacc.Bacc("TRN2", target_bir_lowering=False)

    xhi_t = nc.dram_tensor("xhi", [IMGS_PER_CORE, CIN, WP, WP], F16, kind="ExternalInput")
    xlo_t = nc.dram_tensor("xlo", [IMGS_PER_CORE, CIN, WP, WP], F16, kind="ExternalInput")
    w1s_t = nc.dram_tensor("w1s", [128, 192], F16, kind="ExternalInput")
    w2s_t = nc.dram_tensor("w2s", [128, 576], BF16, kind="ExternalInput")
    b1s_t = nc.dram_tensor("b1s", [128, 1], F32, kind="ExternalInput")
    b2s_t = nc.dram_tensor("b2s", [128, 1], F32, kind="ExternalInput")
    # partition-major output: [128 sbuf partitions, img, (g, s2, px)]
    out_t = nc.dram_tensor("out", [128, IMGS_PER_CORE, 2048], BF16, kind="ExternalOutput")

    xhi_h = xhi_t.ap().tensor
    xlo_h = xlo_t.ap().tensor
    outr = out_t.ap()
    IMG_X = CIN * WP * WP          # elements per image in xhi/xlo

    with tile.TileContext(nc) as tc:
        # ---- persistent SBUF tensors ----
        w1 = nc.alloc_sbuf_tensor("w1sb", [128, 192], F16).ap()
        w2 = nc.alloc_sbuf_tensor("w2sb", [128, 576], BF16).ap()
        b1 = nc.alloc_sbuf_tensor("b1sb", [128, 1], F32).ap()
        b2 = nc.alloc_sbuf_tensor("b2sb", [128, 1], F32).ap()
        t1 = [
            [nc.alloc_sbuf_tensor(f"t1_{p}_{b}", [96, T1_FREE], F16).ap()
             for b in range(nbuf)]
            for p in range(2)
        ]
        hs = [nc.alloc_sbuf_tensor(f"hs_{b}", [128, HS_FREE], BF16).ap()
              for b in range(hbufs)]
        ob = [nc.alloc_sbuf_tensor(f"ob_{b}", [128, 2048], BF16).ap()
              for b in range(2)]

        nc.sync.dma_start(out=w1, in_=w1s_t.ap())
        nc.sync.dma_start(out=w2, in_=w2s_t.ap())
        nc.sync.dma_start(out=b1, in_=b1s_t.ap())
        nc.sync.dma_start(out=b2, in_=b2s_t.ap())
        # h slab borders (col 0/65 of every row, g0 row 0, g1 row 33) must
        # stay zero forever; per-image writes only touch interior cols 1:65
        # of rows 1..32 plus the two halo rows.
        for b in range(hbufs):
            nc.gpsimd.memset(hs[b][:, :], 0.0)

        # conv2 tap orders: boundary taps last.
        # pc (g0): block 3 dy=2 reads img row 32, which lives in g1 row 1.
        # pd (g1): block 4 dy=0 reads img row 31 (g0 row 32, round-3 ACT).
        TAPS_PC = [(dy, dx) for dy in (0, 1, 2) for dx in range(3)]
        TAPS_PD = [(dy, dx) for dy in (1, 2, 0) for dx in range(3)]

        with tc.tile_pool(name="psA", bufs=pa_bufs, space="PSUM") as pool_a, \
             tc.tile_pool(name="psB", bufs=2, space="PSUM") as pool_cd:
            n_iters = IMGS_PER_CORE * reps

            def issue_x_load(iv):
                # one strided DMA per precision builds all 3 dy slabs
                im = iv % IMGS_PER_CORE
                for tt, th in ((t1[0][iv % nbuf], xhi_h),
                               (t1[1][iv % nbuf], xlo_h)):
                    src = _raw_ap(th, im * IMG_X,
                                  [(WP, 3), (WP * WP, CIN), (1, T1_FREE)])
                    nc.sync.dma_start(out=tt[:, :], in_=src)

            # prefetch depth 2
            issue_x_load(0)
            if n_iters > 1:
                issue_x_load(1)

            for img_v in range(n_iters):
                img = img_v % IMGS_PER_CORE
                tb = img_v % nbuf
                hb = img_v % hbufs
                t1h, t1l = t1[0][tb], t1[1][tb]
                if img_v + 2 < n_iters:
                    issue_x_load(img_v + 2)

                tvh = t1h.rearrange("p (h w) -> p h w", w=WP)
                tvl = t1l.rearrange("p (h w) -> p h w", w=WP)
                hv = hs[hb].rearrange("p (r w) -> p r w", w=WP)

                # ---- conv1: 4 rounds x (2 col tiles), K=96 ----
                for r in range(4):
                    pa = pool_a.tile([128, 512], F32, tag="pa")
                    n_mm = 0
                    for dx in range(3):
                        lw = w1[0:96, dx * 64:(dx + 1) * 64]
                        for tv in (tvh, tvl):
                            st = n_mm == 0
                            sp = n_mm == 5
                            nc.tensor.matmul(
                                pa[0:64, :], lw,
                                tv[0:96, 8 * r: 8 * r + 8, dx: dx + 64],
                                start=st, stop=sp, tile_position=(0, 0), skip_group_check=True)
                            nc.tensor.matmul(
                                pa[64:128, :], lw,
                                tv[0:96, 32 + 8 * r: 40 + 8 * r, dx: dx + 64],
                                start=st, stop=sp, tile_position=(0, 64), skip_group_check=True)
                            n_mm += 1
                    # h = sign(conv1 + b1): g0 block r / g1 block r+4 land at
                    # the same slab-row offsets -> one full-lane ACT.
                    nc.scalar.activation(
                        out=hv[:, 1 + 8 * r: 9 + 8 * r, 1:65],
                        in_=pa[:, :].rearrange("p (a b) -> p a b", b=64),
                        func=ACT_SIGN, bias=b1[:, 0:1])
                    bmm_pc = bmm in (True, 2)
                    bmm_pd = bmm in (True, 3)
                    if not bmm_pc and r == 0:
                        nc.sync.dma_start(out=hv[0:64, 33:34, 1:65],
                                          in_=hv[64:128, 1:2, 1:65])
                    if not bmm_pd and r == 3:
                        nc.sync.dma_start(out=hv[64:128, 0:1, 1:65],
                                          in_=hv[0:64, 32:33, 1:65])


                # ---- conv2: 2 super-rounds x 4 concurrent 64x64 tiles ----
                obt = ob[img_v % 2]
                for s2 in range(2):
                    pc = pool_cd.tile([128, 512], F32, tag="pc")
                    pd = pool_cd.tile([128, 512], F32, tag="pd")
                    bA, bB = 2 * s2, 2 * s2 + 1
                    for ti in range(9):
                        st = ti == 0
                        sp = ti == 8
                        dyc, dxc = TAPS_PC[ti]
                        dyd, dxd = TAPS_PD[ti]
                        lwc = w2[0:64, (dyc * 3 + dxc) * 64:(dyc * 3 + dxc + 1) * 64]
                        lwd = w2[64:128, (dyd * 3 + dxd) * 64:(dyd * 3 + dxd + 1) * 64]
                        nc.tensor.matmul(
                            pc[0:64, :], lwc,
                            hv[0:64, 8 * bA + dyc: 8 * bA + dyc + 8, dxc: dxc + 64],
                            start=st, stop=sp, tile_position=(0, 0), skip_group_check=True)
                        if bmm in (True, 2) and s2 == 1 and dyc == 2:
                            # block 3 boundary: img row 32 lives in g1 slab
                            # row 1 -> cross-group 1-row tap for the last 64
                            # pixels, main tap covers rows 26..32 (448 px).
                            nc.tensor.matmul(
                                pc[64:128, 448:512],
                                w2[64:128, (dyc * 3 + dxc) * 64:(dyc * 3 + dxc + 1) * 64],
                                hv[64:128, 1:2, dxc: dxc + 64],
                                start=False, stop=False, tile_position=(64, 64), skip_group_check=True)
                            nc.tensor.matmul(
                                pc[64:128, 0:448], lwc,
                                hv[0:64, 8 * bB + dyc: 8 * bB + dyc + 7, dxc: dxc + 64],
                                start=st, stop=sp, tile_position=(0, 64), skip_group_check=True)
                        else:
                            nc.tensor.matmul(
                                pc[64:128, :], lwc,
                                hv[0:64, 8 * bB + dyc: 8 * bB + dyc + 8, dxc: dxc + 64],
                                start=st, stop=sp, tile_position=(0, 64), skip_group_check=True)
                        if bmm in (True, 3) and s2 == 0 and dyd == 0:
                            # block 4 boundary: img row 31 lives in g0 slab
                            # row 32 -> cross-group 1-row tap for the first
                            # 64 pixels, main tap covers rows 1..7 (448 px).
                            nc.tensor.matmul(
                                pd[0:64, 0:64],
                                w2[0:64, (dyd * 3 + dxd) * 64:(dyd * 3 + dxd + 1) * 64],
                                hv[0:64, 32:33, dxd: dxd + 64],
                                start=False, stop=False, tile_position=(0, 0), skip_group_check=True)
                            nc.tensor.matmul(
                                pd[0:64, 64:512], lwd,
                                hv[64:128, 1:8, dxd: dxd + 64],
                                start=st, stop=sp, tile_position=(64, 0), skip_group_check=True)
                        else:
                            nc.tensor.matmul(
                                pd[0:64, :], lwd,
                                hv[64:128, 8 * bA + dyd: 8 * bA + dyd + 8, dxd: dxd + 64],
                                start=st, stop=sp, tile_position=(64, 0), skip_group_check=True)
                        nc.tensor.matmul(
                            pd[64:128, :], lwd,
                            hv[64:128, 8 * bB + dyd: 8 * bB + dyd + 8, dxd: dxd + 64],
                            start=st, stop=sp, tile_position=(64, 64), skip_group_check=True)
                    # out = psum + sign(b2), both halves on DVE.
                    # obt free layout: [g=2 (1024), s2=2 (512), px (1)]
                    nc.vector.tensor_scalar(
                        out=obt[:, s2 * 512:(s2 + 1) * 512], in0=pc[:, :],
                        scalar1=b2[:, 0:1], scalar2=None, op0=ALU_ADD)
                    nc.vector.tensor_scalar(
                        out=obt[:, 1024 + s2 * 512: 1536 + s2 * 512],
                        in0=pd[:, :],
                        scalar1=b2[:, 0:1], scalar2=None, op0=ALU_ADD)

                # ---- one out DMA per image (sync queue, 128 descs) ----
                if outq == "sync":
                    nc.sync.dma_start(out=outr[:, img, :], in_=obt[:, :])
                else:
                    nc.scalar.dma_start(out=outr[:, img, :], in_=obt[:, :])

    nc.compile()
    return nc


_CACHE: dict = {}


def _get_nc(reps: int = 1, **kw) -> bacc.Bacc:
    key = (reps, tuple(sorted(kw.items())))
    if key not in _CACHE:
        _CACHE[key] = build_nc(reps, **kw)
    return _CACHE[key]


def _sign(a: np.ndarray) -> np.ndarray:
    return np.where(a >= 0, np.float32(1.0), np.float32(-1.0))


def _prep_inputs(x, w1, b1, w2, b2, scale_val):
    x = np.asarray(x, np.float32)
    n = x.shape[0]
    # fp16 hi/lo split (hi+lo carries ~24 mantissa bits of x)
    xhi = x.astype(np.float16)
    xlo = (x - xhi.astype(np.float32)).astype(np.float16)
    xhi_pad = np.zeros((n, CIN, WP, WP), np.float16)
    xlo_pad = np.zeros((n, CIN, WP, WP), np.float16)
    xhi_pad[:, :, 1:65, 1:65] = xhi
    xlo_pad[:, :, 1:65, 1:65] = xlo

    w1b = _sign(np.asarray(w1, np.float32))           # [64o, 32c, 3, 3]
    w2b = _sign(np.asarray(w2, np.float32))           # [64o, 64c, 3, 3]
    w1s = np.zeros((128, 192), np.float16)
    for dx in range(3):
        for dy in range(3):
            w1s[dy * 32:dy * 32 + 32, dx * 64:(dx + 1) * 64] = w1b[:, :, dy, dx].T
    w2s = np.zeros((128, 576), ml_dtypes.bfloat16)
    for dy in range(3):
        for dx in range(3):
            tap = dy * 3 + dx
            blk = w2b[:, :, dy, dx].T.astype(ml_dtypes.bfloat16)
            w2s[0:64, tap * 64:(tap + 1) * 64] = blk
            w2s[64:128, tap * 64:(tap + 1) * 64] = blk
    b1s = np.tile(_sign(np.asarray(b1, np.float32)), 2).reshape(128, 1).astype(np.float32)
    b2s = np.tile(_sign(np.asarray(b2, np.float32)), 2).reshape(128, 1).astype(np.float32)

    per = n // N_CORES
    in_maps = []
    for i in range(N_CORES):
        sl = slice(i * per, (i + 1) * per)
        in_maps.append({
            "xhi": np.ascontiguousarray(xhi_pad[sl]),
            "xlo": np.ascontiguousarray(xlo_pad[sl]),
            "w1s": w1s, "w2s": w2s, "b1s": b1s, "b2s": b2s,
        })
    return in_maps


def kernel(x, w1, b1, w2, b2, scale) -> np.ndarray:
    scale_val = float(np.asarray(scale).reshape(-1)[0])
    nc = _get_nc(reps=1)
    in_maps = _prep_inputs(x, w1, b1, w2, b2, scale_val)
    res = run_bass_kernel_spmd(nc, in_maps, core_ids=list(range(N_CORES)))
    # out HBM layout: [128, img, 2048] where partition p = (p_hi, ch),
    # free = (g, s, px); image block index = g*4 + s*2 + p_hi, image
    # pixel = block*512 + px (row-major 64x64).
    parts = []
    for r in res.results:
        o = np.asarray(r["out"]).reshape(2, 64, IMGS_PER_CORE, 2, 2, 512)
        o = o.transpose(2, 1, 3, 4, 0, 5).reshape(IMGS_PER_CORE, COUT, H, W)
        parts.append(o)
    out = np.concatenate(parts, axis=0)
    return out.astype(np.float32) * np.float32(scale_val)


if __name__ == "__main__":
    rng = np.random.default_rng(0)
    ins = {
        "x": rng.standard_normal((64, 32, 64, 64), dtype=np.float32),
        "w1": (rng.standard_normal((64, 32, 3, 3)) * 0.05).astype(np.float32),
        "b1": (rng.standard_normal((64,)) * 0.05).astype(np.float32),
        "w2": (rng.standard_normal((64, 64, 3, 3)) * 0.05).astype(np.float32),
        "b2": (rng.standard_normal((64,)) * 0.05).astype(np.float32),
        "scale": np.array([0.001], np.float32),
    }
    out = kernel(**ins)
    print("out", out.shape, out.dtype, float(np.abs(out).mean()))


# revision 18
# speedup vs baseline: 1.6119x; 1.0946x over previous
"""Binarized 2-layer conv net (BinaryConv2d -> BinaryTanh -> BinaryConv2d -> Scale)
for Trainium2, data-parallel over the batch dim across 8 NeuronCores.

Math (matching the reference):
    h   = conv2d(x, sign(w1), pad=1) + sign(b1)
    h   = sign(h)                       # sign(clip(h,-1,1)) == sign(h)
    out = (conv2d(h, sign(w2), pad=1) + sign(b2)) * scale

Device mapping (per core, 8 images):
  * x split on host into fp16 hi + fp16 lo (~24 mantissa bits together,
    effectively fp32-exact), pre-padded to 66x66.
  * x load: ONE DMA per precision per image builds the 3 dy-shifted
    overlapping slabs [96, 64*66] directly from HBM via a raw strided AP
    [[66,3],[4356,32],[1,4224]] (overlapping windows; single trigger).
  * conv1: K=96 (3 dy-slabs x 32 cin), dx taps as free-dim offsets.
    Round r computes g0 block r (PE col tile 0) and g1 block r+4 (col
    tile 1); 12 matmuls (3 dx x 2 precisions x 2 col tiles) accumulate
    into one [128,512] PSUM bank; sign(conv1+b1) evacuates in a single
    full-lane ScalarE ACT into the h slab.
  * h layout: big contiguous slab [128, 34*66] bf16. Partitions 0:64
    (g0) hold image rows -1..32 as slab rows 0..33; partitions 64:128
    (g1) hold rows 31..64. Slab rows 0 (g0/g1) and 33 are zero pad or
    unwritten; NO halo DMAs: the two cross-group boundary rows are
    consumed by conv2 directly from the other partition group via
    split tap matmuls (main 7-row N=448 + cross-group 1-row N=64 with
    the transposed tile_position). This keeps DMA entirely off the
    PE-critical path.
  * conv2: bf16, K=64, 4 concurrent 64x64 PE tiles (2 row groups x 2
    col groups) process 4 blocks at once; 9 tap-matmuls each; dy order
    arranged so boundary taps come last. PSUM evac (out = psum +
    sign(b2), exact small ints in bf16) via two DVE tensor_scalar ops
    into a [128, 2048] staging buffer; ONE out DMA per image.
  * HW-DGE descriptor generation costs ~13ns/descriptor ON the
    triggering engine queue and stalls on ring backpressure behind
    earlier bursts, so: descriptor count is the currency (out HBM
    layout is partition-major [128, 8, 2048] = 128 descs/img, host
    reorders to NCHW; x loads are 96 descs each), and ALL DMA triggers
    live on the sync queue (x prefetch issued 2 images ahead, nbuf=3,
    then out drain) where a stall only delays prefetched/buffered
    work. The scalar queue runs only the critical conv1 sign-ACTs.
"""

import numpy as np
import ml_dtypes

import bass_rust
import concourse.bass as bass
import concourse.mybir as mybir
import concourse.tile as tile
from concourse import bacc
from concourse.bass_utils import run_bass_kernel_spmd

F32 = mybir.dt.float32
F16 = mybir.dt.float16
BF16 = mybir.dt.bfloat16

N_CORES = 8
IMGS_PER_CORE = 8
CIN, COUT = 32, 64
H = W = 64
WP = 66                    # padded width
T1_FREE = H * WP           # 4224, one dy-block slab
HS_ROWS = 34               # h slab rows per partition group (img rows -1..32 / 31..64)
HS_FREE = HS_ROWS * WP     # 2244
ACT_SIGN = mybir.ActivationFunctionType.Sign
ACT_IDENT = mybir.ActivationFunctionType.Identity
ALU_ADD = mybir.AluOpType.add


def _raw_ap(tensor_handle, offset, dims):
    """Build an AP with explicit [stride, size] dims (allows overlap)."""
    return bass_rust.AP(tensor=tensor_handle, offset=offset,
                        ap=[[s, n] for s, n in dims])


def build_nc(reps: int = 1, nbuf: int = 3, pa_bufs: int = 2,
             hbufs: int = 2, bmm: bool = True, outq: str = "sync") -> bacc.Bacc:
    nc = bacc.Bacc("TRN2", target_bir_lowering=False)

    xhi_t = nc.dram_tensor("xhi", [IMGS_PER_CORE, CIN, WP, WP], F16, kind="ExternalInput")
    xlo_t = nc.dram_tensor("xlo", [IMGS_PER_CORE, CIN, WP, WP], F16, kind="ExternalInput")
    w1s_t = nc.dram_tensor("w1s", [128, 192], F16, kind="ExternalInput")
    w2s_t = nc.dram_tensor("w2s", [128, 576], BF16, kind="ExternalInput")
    b1s_t = nc.dram_tensor("b1s", [128, 1], F32, kind="ExternalInput")
    b2s_t = nc.dram_tensor("b2s", [128, 1], F32, kind="ExternalInput")
    # partition-major output: [128 sbuf partitions, img, (g, s2, px)]
    out_t = nc.dram_tensor("out", [128, IMGS_PER_CORE, 2048], BF16, kind="ExternalOutput")

    xhi_h = xhi_t.ap().tensor
    xlo_h = xlo_t.ap().tensor
    outr = out_t.ap()
    IMG_X = CIN * WP * WP          # elements per image in xhi/xlo

    with tile.TileContext(nc) as tc:
        # ---- persistent SBUF tensors ----
        w1 = nc.alloc_sbuf_tensor("w1sb", [128, 192], F16).ap()
        w2 = nc.alloc_sbuf_tensor("w2sb", [128, 576], BF16).ap()
        b1 = nc.alloc_sbuf_tensor("b1sb", [128, 1], F32).ap()
        b2 = nc.alloc_sbuf_tensor("b2sb", [128, 1], F32).ap()
        t1 = [
            [nc.alloc_sbuf_tensor(f"t1_{p}_{b}", [96, T1_FREE], F16).ap()
             for b in range(nbuf)]
            for p in range(2)
        ]
        hs = [nc.alloc_sbuf_tensor(f"hs_{b}", [128, HS_FREE], BF16).ap()
              for b in range(hbufs)]
        ob = [nc.alloc_sbuf_tensor(f"ob_{b}", [128, 2048], BF16).ap()
              for b in range(2)]

        nc.sync.dma_start(out=w1, in_=w1s_t.ap())
        nc.sync.dma_start(out=w2, in_=w2s_t.ap())
        nc.sync.dma_start(out=b1, in_=b1s_t.ap())
        nc.sync.dma_start(out=b2, in_=b2s_t.ap())
        # h slab borders (col 0/65 of every row, g0 row 0, g1 row 33) must
        # stay zero forever; per-image writes only touch interior cols 1:65
        # of rows 1..32 plus the two halo rows.
        for b in range(hbufs):
            nc.gpsimd.memset(hs[b][:, :], 0.0)

        # conv2 tap orders: boundary taps last.
        # pc (g0): block 3 dy=2 reads img row 32, which lives in g1 row 1.
        # pd (g1): block 4 dy=0 reads img row 31 (g0 row 32, round-3 ACT).
        TAPS_PC = [(dy, dx) for dy in (0, 1, 2) for dx in range(3)]
        TAPS_PD = [(dy, dx) for dy in (1, 2, 0) for dx in range(3)]

        with tc.tile_pool(name="psA", bufs=pa_bufs, space="PSUM") as pool_a, \
             tc.tile_pool(name="psB", bufs=2, space="PSUM") as pool_cd:
            n_iters = IMGS_PER_CORE * reps

            def issue_x_load(iv):
                # one strided DMA per precision builds all 3 dy slabs
                im = iv % IMGS_PER_CORE
                for tt, th in ((t1[0][iv % nbuf], xhi_h),
                               (t1[1][iv % nbuf], xlo_h)):
                    src = _raw_ap(th, im * IMG_X,
                                  [(WP, 3), (WP * WP, CIN), (1, T1_FREE)])
                    nc.sync.dma_start(out=tt[:, :], in_=src)

            # prefetch depth 2
            issue_x_load(0)
            if n_iters > 1:
                issue_x_load(1)

            for img_v in range(n_iters):
                img = img_v % IMGS_PER_CORE
                tb = img_v % nbuf
                hb = img_v % hbufs
                t1h, t1l = t1[0][tb], t1[1][tb]
                if img_v + 2 < n_iters:
                    issue_x_load(img_v + 2)

                tvh = t1h.rearrange("p (h w) -> p h w", w=WP)
                tvl = t1l.rearrange("p (h w) -> p h w", w=WP)
                hv = hs[hb].rearrange("p (r w) -> p r w", w=WP)

                # ---- conv1: 4 rounds x (2 col tiles), K=96 ----
                for r in range(4):
                    pa = pool_a.tile([128, 512], F32, tag="pa")
                    n_mm = 0
                    for dx in range(3):
                        lw = w1[0:96, dx * 64:(dx + 1) * 64]
                        for tv in (tvh, tvl):
                            st = n_mm == 0
                            sp = n_mm == 5
                            nc.tensor.matmul(
                                pa[0:64, :], lw,
                                tv[0:96, 8 * r: 8 * r + 8, dx: dx + 64],
                                start=st, stop=sp, tile_position=(0, 0), skip_group_check=True)
                            nc.tensor.matmul(
                                pa[64:128, :], lw,
                                tv[0:96, 32 + 8 * r: 40 + 8 * r, dx: dx + 64],
                                start=st, stop=sp, tile_position=(0, 64), skip_group_check=True)
                            n_mm += 1
                    # h = sign(conv1 + b1): g0 block r / g1 block r+4 land at
                    # the same slab-row offsets -> one full-lane ACT.
                    nc.scalar.activation(
                        out=hv[:, 1 + 8 * r: 9 + 8 * r, 1:65],
                        in_=pa[:, :].rearrange("p (a b) -> p a b", b=64),
                        func=ACT_SIGN, bias=b1[:, 0:1])
                    bmm_pc = bmm in (True, 2)
                    bmm_pd = bmm in (True, 3)
                    if not bmm_pc and r == 0:
                        nc.scalar.dma_start(out=hv[0:64, 33:34, 1:65],
                                            in_=hv[64:128, 1:2, 1:65])
                    if not bmm_pd and r == 3:
                        nc.scalar.dma_start(out=hv[64:128, 0:1, 1:65],
                                            in_=hv[0:64, 32:33, 1:65])


                # ---- conv2: 2 super-rounds x 4 concurrent 64x64 tiles ----
                obt = ob[img_v % 2]
                for s2 in range(2):
                    pc = pool_cd.tile([128, 512], F32, tag="pc")
                    pd = pool_cd.tile([128, 512], F32, tag="pd")
                    bA, bB = 2 * s2, 2 * s2 + 1
                    for ti in range(9):
                        st = ti == 0
                        sp = ti == 8
                        dyc, dxc = TAPS_PC[ti]
                        dyd, dxd = TAPS_PD[ti]
                        lwc = w2[0:64, (dyc * 3 + dxc) * 64:(dyc * 3 + dxc + 1) * 64]
                        lwd = w2[64:128, (dyd * 3 + dxd) * 64:(dyd * 3 + dxd + 1) * 64]
                        nc.tensor.matmul(
                            pc[0:64, :], lwc,
                            hv[0:64, 8 * bA + dyc: 8 * bA + dyc + 8, dxc: dxc + 64],
                            start=st, stop=sp, tile_position=(0, 0), skip_group_check=True)
                        if bmm in (True, 2) and s2 == 1 and dyc == 2:
                            # block 3 boundary: img row 32 lives in g1 slab
                            # row 1 -> cross-group 1-row tap for the last 64
                            # pixels, main tap covers rows 26..32 (448 px).
                            nc.tensor.matmul(
                                pc[64:128, 448:512],
                                w2[64:128, (dyc * 3 + dxc) * 64:(dyc * 3 + dxc + 1) * 64],
                                hv[64:128, 1:2, dxc: dxc + 64],
                                start=False, stop=False, tile_position=(64, 64), skip_group_check=True)
                            nc.tensor.matmul(
                                pc[64:128, 0:448], lwc,
                                hv[0:64, 8 * bB + dyc: 8 * bB + dyc + 7, dxc: dxc + 64],
                                start=st, stop=sp, tile_position=(0, 64), skip_group_check=True)
                        else:
                            nc.tensor.matmul(
                                pc[64:128, :], lwc,
                                hv[0:64, 8 * bB + dyc: 8 * bB + dyc + 8, dxc: dxc + 64],
                                start=st, stop=sp, tile_position=(0, 64), skip_group_check=True)
                        if bmm in (True, 3) and s2 == 0 and dyd == 0:
                            # block 4 boundary: img row 31 lives in g0 slab
                            # row 32 -> cross-group 1-row tap for the first
                            # 64 pixels, main tap covers rows 1..7 (448 px).
                            nc.tensor.matmul(
                                pd[0:64, 0:64],
                                w2[0:64, (dyd * 3 + dxd) * 64:(dyd * 3 + dxd + 1) * 64],
                                hv[0:64, 32:33, dxd: dxd + 64],
                                start=False, stop=False, tile_position=(0, 0), skip_group_check=True)
                            nc.tensor.matmul(
                                pd[0:64, 64:512], lwd,
                                hv[64:128, 1:8, dxd: dxd + 64],
                                start=st, stop=sp, tile_position=(64, 0), skip_group_check=True)
                        else:
                            nc.tensor.matmul(
                                pd[0:64, :], lwd,
                                hv[64:128, 8 * bA + dyd: 8 * bA + dyd + 8, dxd: dxd + 64],
                                start=st, stop=sp, tile_position=(64, 0), skip_group_check=True)
                        nc.tensor.matmul(
                            pd[64:128, :], lwd,
                            hv[64:128, 8 * bB + dyd: 8 * bB + dyd + 8, dxd: dxd + 64],
                            start=st, stop=sp, tile_position=(64, 64), skip_group_check=True)
                    # out = psum + sign(b2), both halves on DVE.
                    # obt free layout: [g=2 (1024), s2=2 (512), px (1)]
                    nc.vector.tensor_scalar(
                        out=obt[:, s2 * 512:(s2 + 1) * 512], in0=pc[:, :],
                        scalar1=b2[:, 0:1], scalar2=None, op0=ALU_ADD)
                    nc.vector.tensor_scalar(
                        out=obt[:, 1024 + s2 * 512: 1536 + s2 * 512],
                        in0=pd[:, :],
                        scalar1=b2[:, 0:1], scalar2=None, op0=ALU_ADD)

                # ---- one out DMA per image (sync queue, 128 descs) ----
                if outq == "sync":
                    nc.sync.dma_start(out=outr[:, img, :], in_=obt[:, :])
                else:
                    nc.scalar.dma_start(out=outr[:, img, :], in_=obt[:, :])

    nc.compile()
    return nc


_CACHE: dict = {}


def _get_nc(reps: int = 1, **kw) -> bacc.Bacc:
    key = (reps, tuple(sorted(kw.items())))
    if key not in _CACHE:
        _CACHE[key] = build_nc(reps, **kw)
    return _CACHE[key]


def _sign(a: np.ndarray) -> np.ndarray:
    return np.where(a >= 0, np.float32(1.0), np.float32(-1.0))


def _prep_inputs(x, w1, b1, w2, b2, scale_val):
    x = np.asarray(x, np.float32)
    n = x.shape[0]
    # fp16 hi/lo split (hi+lo carries ~24 mantissa bits of x)
    xhi = x.astype(np.float16)
    xlo = (x - xhi.astype(np.float32)).astype(np.float16)
    xhi_pad = np.zeros((n, CIN, WP, WP), np.float16)
    xlo_pad = np.zeros((n, CIN, WP, WP), np.float16)
    xhi_pad[:, :, 1:65, 1:65] = xhi
    xlo_pad[:, :, 1:65, 1:65] = xlo

    w1b = _sign(np.asarray(w1, np.float32))           # [64o, 32c, 3, 3]
    w2b = _sign(np.asarray(w2, np.float32))           # [64o, 64c, 3, 3]
    w1s = np.zeros((128, 192), np.float16)
    for dx in range(3):
        for dy in range(3):
            w1s[dy * 32:dy * 32 + 32, dx * 64:(dx + 1) * 64] = w1b[:, :, dy, dx].T
    w2s = np.zeros((128, 576), ml_dtypes.bfloat16)
    for dy in range(3):
        for dx in range(3):
            tap = dy * 3 + dx
            blk = w2b[:, :, dy, dx].T.astype(ml_dtypes.bfloat16)
            w2s[0:64, tap * 64:(tap + 1) * 64] = blk
            w2s[64:128, tap * 64:(tap + 1) * 64] = blk
    b1s = np.tile(_sign(np.asarray(b1, np.float32)), 2).reshape(128, 1).astype(np.float32)
    b2s = np.tile(_sign(np.asarray(b2, np.float32)), 2).reshape(128, 1).astype(np.float32)

    per = n // N_CORES
    in_maps = []
    for i in range(N_CORES):
        sl = slice(i * per, (i + 1) * per)
        in_maps.append({
            "xhi": np.ascontiguousarray(xhi_pad[sl]),
            "xlo": np.ascontiguousarray(xlo_pad[sl]),
            "w1s": w1s, "w2s": w2s, "b1s": b1s, "b2s": b2s,
        })
    return in_maps


def kernel(x, w1, b1, w2, b2, scale) -> np.ndarray:
    scale_val = float(np.asarray(scale).reshape(-1)[0])
    nc = _get_nc(reps=1)
    in_maps = _prep_inputs(x, w1, b1, w2, b2, scale_val)
    res = run_bass_kernel_spmd(nc, in_maps, core_ids=list(range(N_CORES)))
    # out HBM layout: [128, img, 2048] where partition p = (p_hi, ch),
    # free = (g, s, px); image block index = g*4 + s*2 + p_hi, image
    # pixel = block*512 + px (row-major 64x64).
    parts = []
    for r in res.results:
        o = np.asarray(r["out"]).reshape(2, 64, IMGS_PER_CORE, 2, 2, 512)
        o = o.transpose(2, 1, 3, 4, 0, 5).reshape(IMGS_PER_CORE, COUT, H, W)
        parts.append(o)
    out = np.concatenate(parts, axis=0)
    return out.astype(np.float32) * np.float32(scale_val)


if __name__ == "__main__":
    rng = np.random.default_rng(0)
    ins = {
        "x": rng.standard_normal((64, 32, 64, 64), dtype=np.float32),
        "w1": (rng.standard_normal((64, 32, 3, 3)) * 0.05).astype(np.float32),
        "b1": (rng.standard_normal((64,)) * 0.05).astype(np.float32),
        "w2": (rng.standard_normal((64, 64, 3, 3)) * 0.05).astype(np.float32),
        "b2": (rng.standard_normal((64,)) * 0.05).astype(np.float32),
        "scale": np.array([0.001], np.float32),
    }
    out = kernel(**ins)
    print("out", out.shape, out.dtype, float(np.abs(out).mean()))
